# revision 8
# baseline (speedup 1.0000x reference)
"""Trainium2 Bass kernel for nn_MultiHeadSelfAttention_29403346108551.

Reference semantics (faithful to the original nn.Module):
  q/k/v = (x @ W.T + b) .reshape(b, 16, 2048, 64)   # reshape, NOT transpose
  RoPE with a *scalar* position t=seq_len (same angle for every token),
  scores = q k^T / 8, softmax, o = p v, merge heads, o @ wo.T + bo.

Structural facts used for sharding:
  - The head split is a row-major reshape: head h reads x rows [128h, 128h+128)
    and ALL 1024 features; within-head time t = r*16 + jc (r = x-row in block,
    jc = feature chunk j//64), d = j%64.  Permuted time t'' = jc*128 + r is
    used on-device; the host un-permutes.
  - RoPE rotation folded into wq/wk/bq/bk on the host (scalar position).
  - Core cid: batch cid//4, head group cid%4 (4 heads = x rows [512g, 512g+512)).
    Output projection partials summed across the 4 cores of a batch on host.

v2 design (vs. baseline):
  - All matmul operands bf16 (PSUM accumulation stays f32); rel-err budget 2e-2.
  - Every matmul is full-array tile_size (128,128): scores use a zero-padded
    interleaved Q layout (A-half: qA in partitions 0-63 / zeros 64-127;
    B-half mirrored) so one [128,128] kT2a chunk (kA rows 0-63, kB rows 64-127)
    serves both heads of a pair with an N=1024 moving operand.  This avoids
    the PE tiling-mode switches (64x128 <-> 128x128) that drained the array
    between every scores/PV group in the baseline.
  - exp split across ACT (native Exp) and DVE (Schraudolph: one tensor_scalar
    f32->int16 whose bits are the bf16 of exp(s/8)); softmax denominator via
    the ones-column in v_aug (row 64 of the PV output).
  - Output projection accumulates BOTH head pairs into one PSUM tile
    (wq-outer loop), halving output DMA; host sums 4 cores per batch.
  - ~18 dummy matmuls at t=0 warm the HAM clock gate during the input DMA.
"""

import numpy as np
import ml_dtypes

import concourse.bass as bass
import concourse.mybir as mybir
import concourse.tile as tile
from concourse import bacc
from concourse.bass_utils import run_bass_kernel_spmd

F32 = mybir.dt.float32
BF16 = mybir.dt.bfloat16
I16 = mybir.dt.int16

MODEL_DIM = 1024
NUM_HEADS = 16
D_K = 64
B = 2
T = 2048
N_CORES = 8
NK = 8              # contraction chunks of 128 over MODEL_DIM
RPC = 512           # x rows per core
SEQ_POS = 2048      # scalar rope position used by the reference

# Schraudolph exp constants: bf16 bits of exp(0.125*s) ~= round(s*SCA + SCB)
SC_A = 0.125 * float(np.log2(np.e)) * 128.0
SC_B = 127.0 * 128.0 - 5.5
DVE_EXP = True      # use DVE int-trick for part of the exp work


def _build_program() -> bass.Bass:
    nc = bacc.Bacc(None, target_bir_lowering=False, debug=False)

    xT = nc.dram_tensor("xT", [MODEL_DIM, RPC], BF16, kind="ExternalInput")
    wqT = nc.dram_tensor("wqT", [MODEL_DIM, MODEL_DIM], BF16, kind="ExternalInput")
    wkT = nc.dram_tensor("wkT", [MODEL_DIM, MODEL_DIM], BF16, kind="ExternalInput")
    wvT = nc.dram_tensor("wvT", [MODEL_DIM, MODEL_DIM], BF16, kind="ExternalInput")
    woT = nc.dram_tensor("woT", [2, 128, MODEL_DIM], BF16, kind="ExternalInput")
    bq = nc.dram_tensor("bq", [128, 8], F32, kind="ExternalInput")
    bk = nc.dram_tensor("bk", [128, 8], F32, kind="ExternalInput")
    bv = nc.dram_tensor("bv", [MODEL_DIM], F32, kind="ExternalInput")
    outp = nc.dram_tensor("outp", [T, MODEL_DIM], F32, kind="ExternalOutput")

    with tile.TileContext(nc) as tc:
        with (
            tc.tile_pool(name="xpool", bufs=8) as xpool,
            tc.tile_pool(name="wpool", bufs=17) as wpool,
            tc.tile_pool(name="cpool", bufs=1) as cpool,
            tc.tile_pool(name="qkpool", bufs=1) as qkpool,
            tc.tile_pool(name="vpool", bufs=4) as vpool,
            tc.tile_pool(name="espool", bufs=4) as espool,
            tc.tile_pool(name="o2pool", bufs=1) as o2pool,
            tc.tile_pool(name="outpool", bufs=4) as outpool,
            tc.tile_pool(name="opool", bufs=2) as opool,
            tc.tile_pool(name="rcpool", bufs=2) as rcpool,
            tc.tile_pool(name="rcbig", bufs=2) as rcbig,
        ):
            # ---- warmup MMs keep the PE busy (HAM warm) during input DMA ----
            warm_w = cpool.tile([128, 512], BF16, name="warm_w")
            nc.vector.memset(warm_w, 0.0)
            with tc.tile_pool(name="pswarm", bufs=1, space="PSUM") as pswarm:
                psw = pswarm.tile([128, 512], F32, name="psw")
                for i in range(18):
                    nc.tensor.matmul(psw, warm_w[:, 0:128], warm_w,
                                     start=True, stop=True)
                # last four give the scheduler a consumer so they are not sunk
                wsink = cpool.tile([1, 16], F32, name="wsink")
                nc.scalar.activation(wsink, psw[0:1, 0:16],
                                     mybir.ActivationFunctionType.Exp, scale=1.0)

            # ---- input loads (xt + wq first: first proj MM needs them) ----
            xt = []
            for k in range(NK):
                t_ = xpool.tile([128, RPC], BF16, tag="xt", name=f"xt_{k}")
                nc.sync.dma_start(out=t_, in_=xT[k * 128:(k + 1) * 128, :])
                xt.append(t_)
            wq_sb, wk_sb = [], []
            for k in range(NK):
                t_ = wpool.tile([128, MODEL_DIM], BF16, tag="w", name=f"wq_{k}")
                nc.scalar.dma_start(out=t_, in_=wqT[k * 128:(k + 1) * 128, :])
                wq_sb.append(t_)
            for k in range(NK):
                t_ = wpool.tile([128, MODEL_DIM], BF16, tag="w", name=f"wk_{k}")
                nc.sync.dma_start(out=t_, in_=wkT[k * 128:(k + 1) * 128, :])
                wk_sb.append(t_)

            bq_sb = cpool.tile([128, 8], F32)
            nc.sync.dma_start(out=bq_sb, in_=bq[:, :])
            bk_sb = cpool.tile([128, 8], F32)
            nc.sync.dma_start(out=bk_sb, in_=bk[:, :])
            bv_bc = cpool.tile([128, MODEL_DIM], F32)
            nc.sync.dma_start(
                out=bv_bc,
                in_=bass.AP(tensor=bv[:].tensor, offset=bv[:].offset,
                            ap=[[0, 128]] + [list(p) for p in bv[:].ap]))

            # persistent activation layouts
            # qT2pad: [pair m (2), wq (4), {A,B} halves (2), 512] columns, bf16.
            #   A half: qA d-vals in partitions 0-63, zeros 64-127; B mirrored.
            qT2pad = qkpool.tile([128, 2 * 4 * 2 * 512], BF16, name="qT2pad")
            nc.vector.memset(qT2pad, 0.0)
            # kT2a: pair m at cols [m*T, (m+1)*T); partition = 64*parity + d;
            #   col = t'' = jc*128 + r
            kT2a = qkpool.tile([128, 2 * T], BF16, name="kT2a")
            o2T = [o2pool.tile([128, T], BF16, name=f"o2T_{i}") for i in range(2)]
            wo_sb = []
            for m_ in range(2):
                t_ = cpool.tile([128, MODEL_DIM], BF16, name=f"wo_{m_}")
                nc.scalar.dma_start(out=t_, in_=woT[m_, :, :])
                wo_sb.append(t_)

            # v_aug[bl]: [128 keys(r), 16 chunks(jc), 65]; col 64 = ones
            v_aug = []
            for bl in range(4):
                va = vpool.tile([128, 16, 65], BF16, tag="va", name=f"v_aug_{bl}")
                nc.vector.memset(va[:, :, 64:65], 1.0)
                v_aug.append(va)

            # preload ACT exp table set early
            warm2 = cpool.tile([1, 16], F32, name="warm2")
            nc.scalar.activation(warm2, warm_w.bitcast(F32)[0:1, 0:16],
                                 mybir.ActivationFunctionType.Exp, scale=1.0)

            # ---- projections ----
            with tc.tile_pool(name="psproj", bufs=8, space="PSUM") as psproj:
                wv_sb = []
                for k in range(NK):
                    t_ = wpool.tile([128, MODEL_DIM], BF16, tag="w", name=f"wv_{k}")
                    nc.scalar.dma_start(out=t_, in_=wvT[k * 128:(k + 1) * 128, :])
                    wv_sb.append(t_)

                # Q and K projections: psq[p] = w[:, 128p:+128].T @ x.T
                # psq[p][64*half+d, b*128+r]: feature j=128p+64half+d of
                # x-row (block b, r); head h = b; jc = 2p+half.
                for w_sb, bias_sb, is_q in ((wq_sb, bq_sb, True),
                                            (wk_sb, bk_sb, False)):
                    psq = [psproj.tile([128, RPC], F32, tag="proj",
                                       name=f"psq_{int(is_q)}_{p}")
                           for p in range(8)]
                    for k in range(NK):
                        for p in range(8):
                            nc.tensor.matmul(
                                psq[p], w_sb[k][:, p * 128:(p + 1) * 128], xt[k],
                                start=(k == 0), stop=(k == NK - 1),
                            )
                    for p in range(8):
                        for half in range(2):
                            jc = 2 * p + half
                            for ph in range(2):
                                # src: psq[p][64half+d, (2m+ph)*128+r], m in {0,1}
                                src3 = psq[p][64 * half:64 * half + 64, ph * 128:] \
                                    .rearrange("p (b r) -> p b r", r=128)[:, 0:3:2, :]
                                if is_q:
                                    base = (2 * (jc // 4) + ph) * 512 + (jc % 4) * 128
                                    dst3 = qT2pad[64 * ph:64 * ph + 64, :].rearrange(
                                        "p (m c) -> p m c", m=2)[:, :, base:base + 128]
                                else:
                                    base = jc * 128
                                    dst3 = kT2a[64 * ph:64 * ph + 64, :].rearrange(
                                        "p (m c) -> p m c", m=2)[:, :, base:base + 128]
                                nc.vector.tensor_scalar_add(
                                    dst3, src3,
                                    bias_sb[64 * half:64 * half + 64, p:p + 1],
                                )

                # V projection: psv[2bl+jw][r, jj] = feature jw*512+jj of
                # x-row (bl, r)
                psv = [psproj.tile([128, RPC], F32, tag="proj", name=f"psv_{i}")
                       for i in range(8)]
                for k in range(NK):
                    for bl in range(4):
                        for jw in range(2):
                            nc.tensor.matmul(
                                psv[2 * bl + jw], xt[k][:, bl * 128:(bl + 1) * 128],
                                wv_sb[k][:, jw * 512:(jw + 1) * 512],
                                start=(k == 0), stop=(k == NK - 1),
                            )
                for bl in range(4):
                    for jw in range(2):
                        nc.vector.tensor_tensor(
                            v_aug[bl][:, 8 * jw:8 * jw + 8, 0:64],
                            psv[2 * bl + jw][:, :].rearrange(
                                "p (cc d) -> p cc d", d=64),
                            bv_bc[:, jw * 512:(jw + 1) * 512].rearrange(
                                "p (cc d) -> p cc d", d=64),
                            mybir.AluOpType.add,
                        )

            # ---- attention ----
            with (
                tc.tile_pool(name="psS", bufs=2, space="PSUM") as psS_pool,
                tc.tile_pool(name="psO", bufs=1, space="PSUM") as psO_pool,
                tc.tile_pool(name="psF", bufs=1, space="PSUM") as psF_pool,
            ):
                state = {}

                def emit_scores(i, wq, m, cc):
                    """scores MM (N=1024, both halves) + exp -> eS tile."""
                    psS = psS_pool.tile([128, 1024], F32, tag="s", bufs=2,
                                        name=f"psS_{i}")
                    lhs = kT2a[:, m * T + cc * 128: m * T + cc * 128 + 128]
                    base = (m * 4 + wq) * 1024
                    # one PSUM bank per MM: N<=512 fp32 out
                    nc.tensor.matmul(psS[:, 0:512], lhs,
                                     qT2pad[:, base:base + 512],
                                     start=True, stop=True)
                    nc.tensor.matmul(psS[:, 512:1024], lhs,
                                     qT2pad[:, base + 512:base + 1024],
                                     start=True, stop=True)
                    eS = espool.tile([128, 1024], I16, tag="es", bufs=4,
                                     name=f"eS_{i}")
                    if DVE_EXP and (i % 16) >= 9:
                        nc.vector.tensor_scalar(
                            eS, psS, float(SC_A), float(SC_B),
                            mybir.AluOpType.mult, mybir.AluOpType.add)
                    else:
                        nc.scalar.activation(
                            eS.bitcast(BF16), psS,
                            mybir.ActivationFunctionType.Exp, scale=0.125)
                    return eS

                def normalize(m, wq, ph, psO):
                    """psO [65,512] -> o2T[m][64ph:+64, wq*512:+512]"""
                    o_sb = opool.tile([65, 512], F32, tag="osb", bufs=2)
                    nc.scalar.copy(o_sb, psO)
                    den_t = rcpool.tile([128, 4], F32, tag="dent", bufs=2)
                    nc.gpsimd.dma_start(
                        out=den_t,
                        in_=o_sb[64:65, :].rearrange("a (p i) -> a p i", p=128))
                    rcp_t = rcpool.tile([128, 4], F32, tag="rcpt", bufs=2)
                    nc.vector.reciprocal(rcp_t, den_t)
                    rcp_flat = rcbig.tile([1, 512], F32, tag="rcpf", bufs=2)
                    nc.gpsimd.dma_start(
                        out=rcp_flat[0:1, :].rearrange("a (p i) -> a p i", p=128),
                        in_=rcp_t)
                    rcp_bc = rcbig.tile([64, 512], F32, tag="rcpb", bufs=2)
                    nc.gpsimd.partition_broadcast(rcp_bc, rcp_flat)
                    nc.vector.tensor_tensor(
                        o2T[m][64 * ph:64 * ph + 64, wq * 512:(wq + 1) * 512],
                        o_sb[0:64, :], rcp_bc, mybir.AluOpType.mult)

                def emit_pv(i, wq, m, cc, eS):
                    if cc == 0:
                        state[(wq, m)] = (
                            psO_pool.tile([65, 512], F32, tag="oA",
                                          name=f"psO_A_{wq}_{m}"),
                            psO_pool.tile([65, 512], F32, tag="oB",
                                          name=f"psO_B_{wq}_{m}"),
                        )
                    psO_A, psO_B = state[(wq, m)]
                    eSb = eS.bitcast(BF16)
                    nc.tensor.matmul(psO_A, v_aug[2 * m][:, cc, :],
                                     eSb[:, 0:512],
                                     start=(cc == 0), stop=(cc == 15))
                    nc.tensor.matmul(psO_B, v_aug[2 * m + 1][:, cc, :],
                                     eSb[:, 512:1024],
                                     start=(cc == 0), stop=(cc == 15))
                    if cc == 15:
                        normalize(m, wq, 0, psO_A)
                        normalize(m, wq, 1, psO_B)

                def emit_final(tt, eng):
                    """outp rows [tt*128, +128): both pairs accumulated."""
                    psF = psF_pool.tile([128, 1024], F32, tag="f",
                                        name=f"psF_{tt}")
                    for m2 in range(2):
                        for jw in range(2):
                            nc.tensor.matmul(
                                psF[:, jw * 512:(jw + 1) * 512],
                                o2T[m2][:, tt * 128:(tt + 1) * 128],
                                wo_sb[m2][:, jw * 512:(jw + 1) * 512],
                                start=(m2 == 0), stop=(m2 == 1),
                            )
                    out_sb = outpool.tile([128, MODEL_DIM], F32, tag="out",
                                          bufs=4)
                    if eng == 0:
                        nc.scalar.copy(out_sb, psF)
                    else:
                        nc.vector.tensor_copy(out_sb, psF)
                    nc.sync.dma_start(
                        out=outp[tt * 128:(tt + 1) * 128, :], in_=out_sb)

                iters = [(wq, m, cc) for wq in range(4) for m in range(2)
                         for cc in range(16)]
                # software pipeline: PV lags scores by 2 so the exp result is
                # ready before the PE reaches the PV matmuls
                pend = []
                nfin = 0
                for i, (wq, m, cc) in enumerate(iters):
                    eS = emit_scores(i, wq, m, cc)
                    pend.append((i, wq, m, cc, eS))
                    if len(pend) > 2:
                        emit_pv(*pend.pop(0))
                    # finals for wq-1 spread through (wq, m=0) iterations
                    if wq >= 1 and m == 0 and cc in (3, 6, 9, 12):
                        emit_final(4 * (wq - 1) + (cc - 3) // 3, nfin % 2)
                        nfin += 1
                for p_ in pend:
                    emit_pv(*p_)
                for j in range(4):
                    emit_final(12 + j, j % 2)

    nc.compile()
    return nc


_NC_CACHE = None


def _get_program():
    global _NC_CACHE
    if _NC_CACHE is None:
        _NC_CACHE = _build_program()
    return _NC_CACHE


def _bf16(a: np.ndarray) -> np.ndarray:
    return np.asarray(a, np.float32).astype(ml_dtypes.bfloat16)


def _host_prep(inputs):
    x = np.asarray(inputs["x"], np.float32)
    wq = np.asarray(inputs["wq"], np.float32)
    wk = np.asarray(inputs["wk"], np.float32)
    wv = np.asarray(inputs["wv"], np.float32)
    wo = np.asarray(inputs["wo"], np.float32)
    bq = np.asarray(inputs["bq"], np.float32)
    bk = np.asarray(inputs["bk"], np.float32)
    bv = np.asarray(inputs["bv"], np.float32)
    rot_cos = np.asarray(inputs["rot_cos"], np.float32)
    rot_sin = np.asarray(inputs["rot_sin"], np.float32)

    cos = rot_cos[SEQ_POS]
    sin = rot_sin[SEQ_POS]

    def rope_fold_w(w):
        wv_ = w.reshape(16, 32, 2, MODEL_DIM)
        ev = wv_[:, :, 0] * cos[None, :, None] - wv_[:, :, 1] * sin[None, :, None]
        od = wv_[:, :, 0] * sin[None, :, None] + wv_[:, :, 1] * cos[None, :, None]
        return np.stack([ev, od], axis=2).reshape(MODEL_DIM, MODEL_DIM)

    def rope_fold_b(b_):
        bv_ = b_.reshape(16, 32, 2)
        ev = bv_[:, :, 0] * cos[None, :] - bv_[:, :, 1] * sin[None, :]
        od = bv_[:, :, 0] * sin[None, :] + bv_[:, :, 1] * cos[None, :]
        return np.stack([ev, od], axis=2).reshape(MODEL_DIM)

    wq_r = rope_fold_w(wq)
    wk_r = rope_fold_w(wk)
    bq_r = rope_fold_b(bq)
    bk_r = rope_fold_b(bk)

    wqT = _bf16(np.ascontiguousarray(wq_r.T))
    wkT = _bf16(np.ascontiguousarray(wk_r.T))
    wvT = _bf16(np.ascontiguousarray(wv.T))
    bq_sb = np.ascontiguousarray(bq_r.reshape(8, 128).T)
    bk_sb = np.ascontiguousarray(bk_r.reshape(8, 128).T)

    in_maps = []
    for cid in range(N_CORES):
        bi, g = cid // 4, cid % 4
        xTc = _bf16(np.ascontiguousarray(x[bi, 512 * g:512 * (g + 1), :].T))
        woTc = np.stack(
            [np.ascontiguousarray(
                wo[:, (4 * g + 2 * m) * 64:(4 * g + 2 * m + 2) * 64].T)
             for m in range(2)])
        in_maps.append({
            "xT": xTc,
            "wqT": wqT, "wkT": wkT, "wvT": wvT,
            "woT": _bf16(woTc),
            "bq": bq_sb, "bk": bk_sb, "bv": bv,
        })
    return in_maps, np.asarray(inputs["bo"], np.float32)


def _gather(results, bo):
    out = np.empty((B, T, MODEL_DIM), np.float32)
    for bi in range(B):
        acc = results[4 * bi]["outp"].astype(np.float32)
        for g in range(1, 4):
            acc = acc + results[4 * bi + g]["outp"]
        # t'' = jc*128 + r  ->  t = r*16 + jc
        acc = acc.reshape(16, 128, MODEL_DIM).transpose(1, 0, 2).reshape(
            T, MODEL_DIM)
        out[bi] = acc + bo[None, :]
    return out


def _run(inputs, trace=False, **kw):
    nc = _get_program()
    in_maps, bo = _host_prep(inputs)
    res = run_bass_kernel_spmd(nc, in_maps, list(range(N_CORES)), trace=trace,
                               **kw)
    return _gather(res.results, bo), res


def kernel(**inputs) -> np.ndarray:
    out, _ = _run(inputs)
    return out


# revision 18
# speedup vs baseline: 1.1245x; 1.1245x over previous
"""Trainium2 Bass kernel for nn_MultiHeadSelfAttention_29403346108551.

Reference semantics (faithful to the original nn.Module):
  q/k/v = (x @ W.T + b) .reshape(b, 16, 2048, 64)   # reshape, NOT transpose
  RoPE with a *scalar* position t=seq_len (same angle for every token),
  scores = q k^T / 8, softmax, o = p v, merge heads, o @ wo.T + bo.

Structural facts used for sharding:
  - The head split is a row-major reshape: head h reads x rows [128h, 128h+128)
    and ALL 1024 features; within-head time t = r*16 + jc (r = x-row in block,
    jc = feature chunk j//64), d = j%64.  Permuted time t'' = jc*128 + r is
    used on-device; the host un-permutes.
  - RoPE rotation folded into wq/wk/bq/bk on the host (scalar position).
  - Core cid: batch cid//4, head group cid%4 (4 heads = x rows [512g, 512g+512)).
    Output projection partials summed across the 4 cores of a batch on host.

v2 design (vs. baseline):
  - All matmul operands bf16 (PSUM accumulation stays f32); rel-err budget 2e-2.
  - Every matmul is full-array tile_size (128,128): scores use a zero-padded
    interleaved Q layout (A-half: qA in partitions 0-63 / zeros 64-127;
    B-half mirrored) so one [128,128] kT2a chunk (kA rows 0-63, kB rows 64-127)
    serves both heads of a pair with an N=1024 moving operand.  This avoids
    the PE tiling-mode switches (64x128 <-> 128x128) that drained the array
    between every scores/PV group in the baseline.
  - exp split across ACT (native Exp) and DVE (Schraudolph: one tensor_scalar
    f32->int16 whose bits are the bf16 of exp(s/8)); softmax denominator via
    the ones-column in v_aug (row 64 of the PV output).
  - Output projection accumulates BOTH head pairs into one PSUM tile
    (wq-outer loop), halving output DMA; host sums 4 cores per batch.
  - ~18 dummy matmuls at t=0 warm the HAM clock gate during the input DMA.
"""

import numpy as np
import ml_dtypes

import concourse.bass as bass
import concourse.mybir as mybir
import concourse.tile as tile
from concourse import bacc
from concourse.bass_utils import run_bass_kernel_spmd

F32 = mybir.dt.float32
BF16 = mybir.dt.bfloat16
I16 = mybir.dt.int16

MODEL_DIM = 1024
NUM_HEADS = 16
D_K = 64
B = 2
T = 2048
N_CORES = 8
NK = 8              # contraction chunks of 128 over MODEL_DIM
RPC = 512           # x rows per core
SEQ_POS = 2048      # scalar rope position used by the reference

# Schraudolph exp constants: bf16 bits of exp(0.125*s) ~= round(s*SCA + SCB)
SC_A = 0.125 * float(np.log2(np.e)) * 128.0
SC_B = 127.0 * 128.0 - 5.5
# The Schraudolph DVE exp (one tensor_scalar, +-3.3% per weight) measures
# 1.9e-2 end-to-end rel err on its own (semi-concentrated softmax rows
# amplify it) -- too close to the 2e-2 gate.  exp therefore runs entirely
# on ACT; its ~1.0us/tile matches the PE's ~1.0us/iter, so this costs
# nothing on the critical path.
DVE_EXP = False


def _build_program() -> bass.Bass:
    nc = bacc.Bacc(None, target_bir_lowering=False, debug=False)

    xT = nc.dram_tensor("xT", [MODEL_DIM, RPC], BF16, kind="ExternalInput")
    wqT = nc.dram_tensor("wqT", [MODEL_DIM, MODEL_DIM], BF16, kind="ExternalInput")
    wkT = nc.dram_tensor("wkT", [MODEL_DIM, MODEL_DIM], BF16, kind="ExternalInput")
    wvT = nc.dram_tensor("wvT", [MODEL_DIM, MODEL_DIM], BF16, kind="ExternalInput")
    woT = nc.dram_tensor("woT", [2, 128, MODEL_DIM], BF16, kind="ExternalInput")
    bq = nc.dram_tensor("bq", [128, 8], F32, kind="ExternalInput")
    bk = nc.dram_tensor("bk", [128, 8], F32, kind="ExternalInput")
    bv = nc.dram_tensor("bv", [MODEL_DIM], F32, kind="ExternalInput")
    outp = nc.dram_tensor("outp", [T, MODEL_DIM], F32, kind="ExternalOutput")

    with tile.TileContext(nc) as tc:
        with (
            tc.tile_pool(name="xpool", bufs=8) as xpool,
            tc.tile_pool(name="wpool", bufs=17) as wpool,
            tc.tile_pool(name="cpool", bufs=1) as cpool,
            tc.tile_pool(name="qkpool", bufs=1) as qkpool,
            tc.tile_pool(name="vpool", bufs=4) as vpool,
            tc.tile_pool(name="espool", bufs=4) as espool,
            tc.tile_pool(name="o2pool", bufs=1) as o2pool,
            tc.tile_pool(name="outpool", bufs=4) as outpool,
            tc.tile_pool(name="opool", bufs=2) as opool,
            tc.tile_pool(name="rcpool", bufs=2) as rcpool,
            tc.tile_pool(name="rcbig", bufs=2) as rcbig,
        ):
            # ---- warmup MMs keep the PE busy (HAM warm) during input DMA ----
            warm_w = cpool.tile([128, 512], BF16, name="warm_w")
            nc.vector.memset(warm_w, 0.0)
            with tc.tile_pool(name="pswarm", bufs=1, space="PSUM") as pswarm:
                psw = pswarm.tile([128, 512], F32, name="psw")
                for i in range(18):
                    nc.tensor.matmul(psw, warm_w[:, 0:128], warm_w,
                                     start=True, stop=True)
                # last four give the scheduler a consumer so they are not sunk
                wsink = cpool.tile([1, 16], F32, name="wsink")
                nc.scalar.activation(wsink, psw[0:1, 0:16],
                                     mybir.ActivationFunctionType.Exp, scale=1.0)

            # ---- input loads (xt + wq first: first proj MM needs them) ----
            xt = []
            for k in range(NK):
                t_ = xpool.tile([128, RPC], BF16, tag="xt", name=f"xt_{k}")
                nc.sync.dma_start(out=t_, in_=xT[k * 128:(k + 1) * 128, :])
                xt.append(t_)
            wq_sb, wk_sb = [], []
            for k in range(NK):
                t_ = wpool.tile([128, MODEL_DIM], BF16, tag="w", name=f"wq_{k}")
                nc.scalar.dma_start(out=t_, in_=wqT[k * 128:(k + 1) * 128, :])
                wq_sb.append(t_)
            for k in range(NK):
                t_ = wpool.tile([128, MODEL_DIM], BF16, tag="w", name=f"wk_{k}")
                nc.sync.dma_start(out=t_, in_=wkT[k * 128:(k + 1) * 128, :])
                wk_sb.append(t_)

            bq_sb = cpool.tile([128, 8], F32)
            nc.sync.dma_start(out=bq_sb, in_=bq[:, :])
            bk_sb = cpool.tile([128, 8], F32)
            nc.sync.dma_start(out=bk_sb, in_=bk[:, :])
            bv_bc = cpool.tile([128, MODEL_DIM], F32)
            nc.sync.dma_start(
                out=bv_bc,
                in_=bass.AP(tensor=bv[:].tensor, offset=bv[:].offset,
                            ap=[[0, 128]] + [list(p) for p in bv[:].ap]))

            # persistent activation layouts
            # qT2pad: [pair m (2), wq (4), {A,B} halves (2), 512] columns, bf16.
            #   A half: qA d-vals in partitions 0-63, zeros 64-127; B mirrored.
            qT2pad = qkpool.tile([128, 2 * 4 * 2 * 512], BF16, name="qT2pad")
            nc.vector.memset(qT2pad, 0.0)
            # kT2a: pair m at cols [m*T, (m+1)*T); partition = 64*parity + d;
            #   col = t'' = jc*128 + r
            kT2a = qkpool.tile([128, 2 * T], BF16, name="kT2a")
            o2T = [o2pool.tile([128, T], BF16, name=f"o2T_{i}") for i in range(2)]
            wo_sb = []
            for m_ in range(2):
                t_ = cpool.tile([128, MODEL_DIM], BF16, name=f"wo_{m_}")
                nc.scalar.dma_start(out=t_, in_=woT[m_, :, :])
                wo_sb.append(t_)

            # v_aug[bl]: [128 keys(r), 16 chunks(jc), 65]; col 64 = ones
            v_aug = []
            for bl in range(4):
                va = vpool.tile([128, 16, 65], BF16, tag="va", name=f"v_aug_{bl}")
                nc.vector.memset(va[:, :, 64:65], 1.0)
                v_aug.append(va)

            # preload ACT exp table set early
            warm2 = cpool.tile([1, 16], F32, name="warm2")
            nc.scalar.activation(warm2, warm_w.bitcast(F32)[0:1, 0:16],
                                 mybir.ActivationFunctionType.Exp, scale=1.0)

            # ---- projections ----
            with tc.tile_pool(name="psproj", bufs=8, space="PSUM") as psproj:
                wv_sb = []
                for k in range(NK):
                    t_ = wpool.tile([128, MODEL_DIM], BF16, tag="w", name=f"wv_{k}")
                    nc.scalar.dma_start(out=t_, in_=wvT[k * 128:(k + 1) * 128, :])
                    wv_sb.append(t_)

                # Q and K projections: psq[p] = w[:, 128p:+128].T @ x.T
                # psq[p][64*half+d, b*128+r]: feature j=128p+64half+d of
                # x-row (block b, r); head h = b; jc = 2p+half.
                # p-outer: each psq[p] finishes its k-accumulation and drains
                # while the PE moves on, so Q/K/V matmul streams stay dense
                for w_sb, bias_sb, is_q in ((wq_sb, bq_sb, True),
                                            (wk_sb, bk_sb, False)):
                    psq = {}
                    for p in range(8):
                        psq[p] = psproj.tile([128, RPC], F32, tag="proj",
                                             name=f"psq_{int(is_q)}_{p}")
                        for k in range(NK):
                            nc.tensor.matmul(
                                psq[p], w_sb[k][:, p * 128:(p + 1) * 128], xt[k],
                                start=(k == 0), stop=(k == NK - 1),
                            )
                        for half in range(2):
                            jc = 2 * p + half
                            for ph in range(2):
                                # src: psq[p][64half+d, (2m+ph)*128+r], m in {0,1}
                                src3 = psq[p][64 * half:64 * half + 64, ph * 128:] \
                                    .rearrange("p (b r) -> p b r", r=128)[:, 0:3:2, :]
                                if is_q:
                                    base = (2 * (jc // 4) + ph) * 512 + (jc % 4) * 128
                                    dst3 = qT2pad[64 * ph:64 * ph + 64, :].rearrange(
                                        "p (m c) -> p m c", m=2)[:, :, base:base + 128]
                                else:
                                    base = jc * 128
                                    dst3 = kT2a[64 * ph:64 * ph + 64, :].rearrange(
                                        "p (m c) -> p m c", m=2)[:, :, base:base + 128]
                                nc.vector.tensor_scalar_add(
                                    dst3, src3,
                                    bias_sb[64 * half:64 * half + 64, p:p + 1],
                                )

                # V projection, bl-outer so v_aug[bl] drains progressively:
                # psv[2bl+jw][r, jj] = feature jw*512+jj of x-row (bl, r)
                for bl in range(4):
                    psv = [psproj.tile([128, RPC], F32, tag="proj",
                                       name=f"psv_{bl}_{i}") for i in range(2)]
                    for k in range(NK):
                        for jw in range(2):
                            nc.tensor.matmul(
                                psv[jw], xt[k][:, bl * 128:(bl + 1) * 128],
                                wv_sb[k][:, jw * 512:(jw + 1) * 512],
                                start=(k == 0), stop=(k == NK - 1),
                            )
                    for jw in range(2):
                        nc.vector.tensor_tensor(
                            v_aug[bl][:, 8 * jw:8 * jw + 8, 0:64],
                            psv[jw][:, :].rearrange(
                                "p (cc d) -> p cc d", d=64),
                            bv_bc[:, jw * 512:(jw + 1) * 512].rearrange(
                                "p (cc d) -> p cc d", d=64),
                            mybir.AluOpType.add,
                        )

            # ---- attention ----
            with (
                tc.tile_pool(name="psS", bufs=2, space="PSUM") as psS_pool,
                tc.tile_pool(name="psO", bufs=1, space="PSUM") as psO_pool,
                tc.tile_pool(name="psF", bufs=1, space="PSUM") as psF_pool,
            ):
                state = {}

                def emit_scores(i, wq, m, cc):
                    """scores MM (N=1024, both halves) + exp -> eS tile."""
                    psS = psS_pool.tile([128, 1024], F32, tag="s", bufs=2,
                                        name=f"psS_{i}")
                    lhs = kT2a[:, m * T + cc * 128: m * T + cc * 128 + 128]
                    base = (m * 4 + wq) * 1024
                    # one PSUM bank per MM: N<=512 fp32 out
                    nc.tensor.matmul(psS[:, 0:512], lhs,
                                     qT2pad[:, base:base + 512],
                                     start=True, stop=True)
                    nc.tensor.matmul(psS[:, 512:1024], lhs,
                                     qT2pad[:, base + 512:base + 1024],
                                     start=True, stop=True)
                    eS = espool.tile([128, 1024], I16, tag="es", bufs=4,
                                     name=f"eS_{i}")
                    if DVE_EXP and (i % 2) == 1:
                        nc.vector.tensor_scalar(
                            eS, psS, float(SC_A), float(SC_B),
                            mybir.AluOpType.mult, mybir.AluOpType.add)
                    else:
                        nc.scalar.activation(
                            eS.bitcast(BF16), psS,
                            mybir.ActivationFunctionType.Exp, scale=0.125)
                    return eS

                def normalize(m, wq, ph, psO):
                    """psO [65,512] -> o2T[m][64ph:+64, wq*512:+512].
                    A (ph=0) and B (ph=1) chains run on different engines so
                    they overlap."""
                    o_sb = opool.tile([65, 512], F32, tag="osb", bufs=2)
                    nc.vector.tensor_copy(o_sb, psO)
                    den_t = rcpool.tile([128, 4], F32, tag="dent", bufs=2)
                    nc.gpsimd.dma_start(
                        out=den_t,
                        in_=o_sb[64:65, :].rearrange("a (p i) -> a p i", p=128))
                    rcp_t = rcpool.tile([128, 4], F32, tag="rcpt", bufs=2)
                    nc.vector.reciprocal(rcp_t, den_t)
                    rcp_flat = rcbig.tile([1, 512], F32, tag="rcpf", bufs=2)
                    nc.gpsimd.dma_start(
                        out=rcp_flat[0:1, :].rearrange("a (p i) -> a p i", p=128),
                        in_=rcp_t)
                    rcp_bc = rcbig.tile([64, 512], F32, tag="rcpb", bufs=2)
                    nc.gpsimd.partition_broadcast(rcp_bc, rcp_flat)
                    nc.vector.tensor_tensor(
                        o2T[m][64 * ph:64 * ph + 64, wq * 512:(wq + 1) * 512],
                        o_sb[0:64, :], rcp_bc, mybir.AluOpType.mult)

                def emit_pv(i, wq, m, cc, eS):
                    if cc == 0:
                        state[(wq, m)] = (
                            psO_pool.tile([65, 512], F32, tag="oA",
                                          name=f"psO_A_{wq}_{m}"),
                            psO_pool.tile([65, 512], F32, tag="oB",
                                          name=f"psO_B_{wq}_{m}"),
                        )
                    psO_A, psO_B = state[(wq, m)]
                    eSb = eS.bitcast(BF16)
                    nc.tensor.matmul(psO_A, v_aug[2 * m][:, cc, :],
                                     eSb[:, 0:512],
                                     start=(cc == 0), stop=(cc == 15))
                    nc.tensor.matmul(psO_B, v_aug[2 * m + 1][:, cc, :],
                                     eSb[:, 512:1024],
                                     start=(cc == 0), stop=(cc == 15))
                    if cc == 15:
                        normalize(m, wq, 0, psO_A)
                        normalize(m, wq, 1, psO_B)

                def emit_final(tt, eng):
                    """outp rows [tt*128, +128): both pairs accumulated.
                    Two 1-bank psF halves; the two PSUM->SBUF copies run on
                    ACT and DVE in parallel and pipeline against the MMs."""
                    psh = [psF_pool.tile([128, 512], F32, tag=f"f{j}",
                                         name=f"psF_{tt}_{j}") for j in range(2)]
                    for jw in range(2):
                        for m2 in range(2):
                            nc.tensor.matmul(
                                psh[jw],
                                o2T[m2][:, tt * 128:(tt + 1) * 128],
                                wo_sb[m2][:, jw * 512:(jw + 1) * 512],
                                start=(m2 == 0), stop=(m2 == 1),
                            )
                    out_sb = outpool.tile([128, MODEL_DIM], F32, tag="out",
                                          bufs=4)
                    nc.vector.tensor_copy(out_sb[:, 0:512], psh[0])
                    nc.vector.tensor_copy(out_sb[:, 512:1024], psh[1])
                    nc.sync.dma_start(
                        out=outp[tt * 128:(tt + 1) * 128, :], in_=out_sb)

                iters = [(wq, m, cc) for wq in range(4) for m in range(2)
                         for cc in range(16)]
                # software pipeline: PV lags scores by 2 so the exp result is
                # ready before the PE reaches the PV matmuls
                pend = []
                nfin = 0
                for i, (wq, m, cc) in enumerate(iters):
                    eS = emit_scores(i, wq, m, cc)
                    pend.append((i, wq, m, cc, eS))
                    if len(pend) > 2:
                        emit_pv(*pend.pop(0))
                    # finals for wq-1 spread with slack for the normalize chain
                    if wq >= 1 and ((m == 0 and cc in (6, 10, 14)) or
                                    (m == 1 and cc == 2)):
                        emit_final(4 * (wq - 1) + (nfin % 4), nfin % 2)
                        nfin += 1
                for p_ in pend:
                    emit_pv(*p_)
                for j in range(4):
                    emit_final(12 + j, j % 2)

    nc.compile()
    return nc


_NC_CACHE = None


def _get_program():
    global _NC_CACHE
    if _NC_CACHE is None:
        _NC_CACHE = _build_program()
    return _NC_CACHE


def _bf16(a: np.ndarray) -> np.ndarray:
    return np.asarray(a, np.float32).astype(ml_dtypes.bfloat16)


def _host_prep(inputs):
    x = np.asarray(inputs["x"], np.float32)
    wq = np.asarray(inputs["wq"], np.float32)
    wk = np.asarray(inputs["wk"], np.float32)
    wv = np.asarray(inputs["wv"], np.float32)
    wo = np.asarray(inputs["wo"], np.float32)
    bq = np.asarray(inputs["bq"], np.float32)
    bk = np.asarray(inputs["bk"], np.float32)
    bv = np.asarray(inputs["bv"], np.float32)
    rot_cos = np.asarray(inputs["rot_cos"], np.float32)
    rot_sin = np.asarray(inputs["rot_sin"], np.float32)

    cos = rot_cos[SEQ_POS]
    sin = rot_sin[SEQ_POS]

    def rope_fold_w(w):
        wv_ = w.reshape(16, 32, 2, MODEL_DIM)
        ev = wv_[:, :, 0] * cos[None, :, None] - wv_[:, :, 1] * sin[None, :, None]
        od = wv_[:, :, 0] * sin[None, :, None] + wv_[:, :, 1] * cos[None, :, None]
        return np.stack([ev, od], axis=2).reshape(MODEL_DIM, MODEL_DIM)

    def rope_fold_b(b_):
        bv_ = b_.reshape(16, 32, 2)
        ev = bv_[:, :, 0] * cos[None, :] - bv_[:, :, 1] * sin[None, :]
        od = bv_[:, :, 0] * sin[None, :] + bv_[:, :, 1] * cos[None, :]
        return np.stack([ev, od], axis=2).reshape(MODEL_DIM)

    wq_r = rope_fold_w(wq)
    wk_r = rope_fold_w(wk)
    bq_r = rope_fold_b(bq)
    bk_r = rope_fold_b(bk)

    wqT = _bf16(np.ascontiguousarray(wq_r.T))
    wkT = _bf16(np.ascontiguousarray(wk_r.T))
    wvT = _bf16(np.ascontiguousarray(wv.T))
    bq_sb = np.ascontiguousarray(bq_r.reshape(8, 128).T)
    bk_sb = np.ascontiguousarray(bk_r.reshape(8, 128).T)

    in_maps = []
    for cid in range(N_CORES):
        bi, g = cid // 4, cid % 4
        xTc = _bf16(np.ascontiguousarray(x[bi, 512 * g:512 * (g + 1), :].T))
        woTc = np.stack(
            [np.ascontiguousarray(
                wo[:, (4 * g + 2 * m) * 64:(4 * g + 2 * m + 2) * 64].T)
             for m in range(2)])
        in_maps.append({
            "xT": xTc,
            "wqT": wqT, "wkT": wkT, "wvT": wvT,
            "woT": _bf16(woTc),
            "bq": bq_sb, "bk": bk_sb, "bv": bv,
        })
    return in_maps, np.asarray(inputs["bo"], np.float32)


def _gather(results, bo):
    out = np.empty((B, T, MODEL_DIM), np.float32)
    for bi in range(B):
        acc = results[4 * bi]["outp"].astype(np.float32)
        for g in range(1, 4):
            acc = acc + results[4 * bi + g]["outp"]
        # t'' = jc*128 + r  ->  t = r*16 + jc
        acc = acc.reshape(16, 128, MODEL_DIM).transpose(1, 0, 2).reshape(
            T, MODEL_DIM)
        out[bi] = acc + bo[None, :]
    return out


def _run(inputs, trace=False, **kw):
    nc = _get_program()
    in_maps, bo = _host_prep(inputs)
    res = run_bass_kernel_spmd(nc, in_maps, list(range(N_CORES)), trace=trace,
                               **kw)
    return _gather(res.results, bo), res


def kernel(**inputs) -> np.ndarray:
    out, _ = _run(inputs)
    return out


# revision 24
# speedup vs baseline: 1.1757x; 1.0455x over previous
"""Trainium2 Bass kernel for nn_MultiHeadSelfAttention_29403346108551.

Reference semantics (faithful to the original nn.Module):
  q/k/v = (x @ W.T + b) .reshape(b, 16, 2048, 64)   # reshape, NOT transpose
  RoPE with a *scalar* position t=seq_len (same angle for every token),
  scores = q k^T / 8, softmax, o = p v, merge heads, o @ wo.T + bo.

Structural facts used for sharding:
  - The head split is a row-major reshape: head h reads x rows [128h, 128h+128)
    and ALL 1024 features; within-head time t = r*16 + jc (r = x-row in block,
    jc = feature chunk j//64), d = j%64.  Permuted time t'' = jc*128 + r is
    used on-device; the host un-permutes.
  - RoPE rotation folded into wq/wk/bq/bk on the host (scalar position).
  - Core cid: batch cid//4, head group cid%4 (4 heads = x rows [512g, 512g+512)).
    Output projection partials summed across the 4 cores of a batch on host.

v2 design (vs. baseline):
  - All matmul operands bf16 (PSUM accumulation stays f32); rel-err budget 2e-2.
  - Every matmul is full-array tile_size (128,128): scores use a zero-padded
    interleaved Q layout (A-half: qA in partitions 0-63 / zeros 64-127;
    B-half mirrored) so one [128,128] kT2a chunk (kA rows 0-63, kB rows 64-127)
    serves both heads of a pair with an N=1024 moving operand.  This avoids
    the PE tiling-mode switches (64x128 <-> 128x128) that drained the array
    between every scores/PV group in the baseline.
  - exp split across ACT (native Exp) and DVE (Schraudolph: one tensor_scalar
    f32->int16 whose bits are the bf16 of exp(s/8)); softmax denominator via
    the ones-column in v_aug (row 64 of the PV output).
  - Output projection accumulates BOTH head pairs into one PSUM tile
    (wq-outer loop), halving output DMA; host sums 4 cores per batch.
  - ~18 dummy matmuls at t=0 warm the HAM clock gate during the input DMA.
"""

import numpy as np
import ml_dtypes

import concourse.bass as bass
import concourse.mybir as mybir
import concourse.tile as tile
from concourse import bacc
from concourse.bass_utils import run_bass_kernel_spmd

F32 = mybir.dt.float32
BF16 = mybir.dt.bfloat16
I16 = mybir.dt.int16

MODEL_DIM = 1024
NUM_HEADS = 16
D_K = 64
B = 2
T = 2048
N_CORES = 8
NK = 8              # contraction chunks of 128 over MODEL_DIM
RPC = 512           # x rows per core
SEQ_POS = 2048      # scalar rope position used by the reference

# Schraudolph exp constants: bf16 bits of exp(0.125*s) ~= round(s*SCA + SCB)
SC_A = 0.125 * float(np.log2(np.e)) * 128.0
SC_B = 127.0 * 128.0 - 5.5
# The Schraudolph DVE exp (one tensor_scalar, +-3.3% per weight) measures
# 1.9e-2 end-to-end rel err on its own (semi-concentrated softmax rows
# amplify it) -- too close to the 2e-2 gate.  exp therefore runs entirely
# on ACT; its ~1.0us/tile matches the PE's ~1.0us/iter, so this costs
# nothing on the critical path.
DVE_EXP = False


def _build_program() -> bass.Bass:
    nc = bacc.Bacc(None, target_bir_lowering=False, debug=False)

    xT = nc.dram_tensor("xT", [MODEL_DIM, RPC], BF16, kind="ExternalInput")
    wqT = nc.dram_tensor("wqT", [MODEL_DIM, MODEL_DIM], BF16, kind="ExternalInput")
    wkT = nc.dram_tensor("wkT", [MODEL_DIM, MODEL_DIM], BF16, kind="ExternalInput")
    wvT = nc.dram_tensor("wvT", [MODEL_DIM, MODEL_DIM], BF16, kind="ExternalInput")
    woT = nc.dram_tensor("woT", [2, 128, MODEL_DIM], BF16, kind="ExternalInput")
    bq = nc.dram_tensor("bq", [128, 8], F32, kind="ExternalInput")
    bk = nc.dram_tensor("bk", [128, 8], F32, kind="ExternalInput")
    bv = nc.dram_tensor("bv", [MODEL_DIM], F32, kind="ExternalInput")
    outp = nc.dram_tensor("outp", [T, MODEL_DIM], F32, kind="ExternalOutput")

    with tile.TileContext(nc) as tc:
        with (
            tc.tile_pool(name="xpool", bufs=8) as xpool,
            tc.tile_pool(name="wpool", bufs=17) as wpool,
            tc.tile_pool(name="cpool", bufs=1) as cpool,
            tc.tile_pool(name="qkpool", bufs=1) as qkpool,
            tc.tile_pool(name="vpool", bufs=4) as vpool,
            tc.tile_pool(name="espool", bufs=4) as espool,
            tc.tile_pool(name="o2pool", bufs=1) as o2pool,
            tc.tile_pool(name="outpool", bufs=4) as outpool,
            tc.tile_pool(name="opool", bufs=2) as opool,
            tc.tile_pool(name="rcpool", bufs=2) as rcpool,
            tc.tile_pool(name="rcbig", bufs=2) as rcbig,
        ):
            # ---- warmup MMs keep the PE busy (HAM warm) during input DMA ----
            warm_w = cpool.tile([128, 512], BF16, name="warm_w")
            nc.vector.memset(warm_w, 0.0)
            with tc.tile_pool(name="pswarm", bufs=1, space="PSUM") as pswarm:
                psw = pswarm.tile([128, 512], F32, name="psw")
                for i in range(18):
                    nc.tensor.matmul(psw, warm_w[:, 0:128], warm_w,
                                     start=True, stop=True)
                # last four give the scheduler a consumer so they are not sunk
                wsink = cpool.tile([1, 16], F32, name="wsink")
                nc.scalar.activation(wsink, psw[0:1, 0:16],
                                     mybir.ActivationFunctionType.Exp, scale=1.0)

            # ---- input loads (xt + wq first: first proj MM needs them) ----
            xt = []
            for k in range(NK):
                t_ = xpool.tile([128, RPC], BF16, tag="xt", name=f"xt_{k}")
                nc.sync.dma_start(out=t_, in_=xT[k * 128:(k + 1) * 128, :])
                xt.append(t_)
            wq_sb, wk_sb = [], []
            for k in range(NK):
                t_ = wpool.tile([128, MODEL_DIM], BF16, tag="w", name=f"wq_{k}")
                # split across two queues: p-outer Q-proj needs all 8 chunks
                # almost immediately
                eng = nc.scalar if k % 2 == 0 else nc.gpsimd
                eng.dma_start(out=t_, in_=wqT[k * 128:(k + 1) * 128, :])
                wq_sb.append(t_)
            for k in range(NK):
                t_ = wpool.tile([128, MODEL_DIM], BF16, tag="w", name=f"wk_{k}")
                nc.sync.dma_start(out=t_, in_=wkT[k * 128:(k + 1) * 128, :])
                wk_sb.append(t_)

            bq_sb = cpool.tile([128, 8], F32)
            nc.sync.dma_start(out=bq_sb, in_=bq[:, :])
            bk_sb = cpool.tile([128, 8], F32)
            nc.sync.dma_start(out=bk_sb, in_=bk[:, :])
            bv_bc = cpool.tile([128, MODEL_DIM], F32)
            nc.sync.dma_start(
                out=bv_bc,
                in_=bass.AP(tensor=bv[:].tensor, offset=bv[:].offset,
                            ap=[[0, 128]] + [list(p) for p in bv[:].ap]))

            # persistent activation layouts
            # qT2pad: [pair m (2), wq (4), {A,B} halves (2), 512] columns, bf16.
            #   A half: qA d-vals in partitions 0-63, zeros 64-127; B mirrored.
            qT2pad = qkpool.tile([128, 2 * 4 * 2 * 512], BF16, name="qT2pad")
            nc.vector.memset(qT2pad, 0.0)
            # kT2a: pair m at cols [m*T, (m+1)*T); partition = 64*parity + d;
            #   col = t'' = jc*128 + r
            kT2a = qkpool.tile([128, 2 * T], BF16, name="kT2a")
            o2T = [o2pool.tile([128, T], BF16, name=f"o2T_{i}") for i in range(2)]
            wo_sb = []
            for m_ in range(2):
                t_ = cpool.tile([128, MODEL_DIM], BF16, name=f"wo_{m_}")
                nc.scalar.dma_start(out=t_, in_=woT[m_, :, :])
                wo_sb.append(t_)

            # v_aug[bl]: [128 keys(r), 16 chunks(jc), 65]; col 64 = ones
            v_aug = []
            for bl in range(4):
                va = vpool.tile([128, 16, 65], BF16, tag="va", name=f"v_aug_{bl}")
                nc.vector.memset(va[:, :, 64:65], 1.0)
                v_aug.append(va)

            # preload ACT exp table set early
            warm2 = cpool.tile([1, 16], F32, name="warm2")
            nc.scalar.activation(warm2, warm_w.bitcast(F32)[0:1, 0:16],
                                 mybir.ActivationFunctionType.Exp, scale=1.0)
            # pre-warm the gpsimd dma<->broadcast ucode libraries: the first
            # switch costs a ~7us library reload if taken mid-attention
            gwa = cpool.tile([1, 16], F32, name="gwa")
            gwb = cpool.tile([4, 16], F32, name="gwb")
            nc.gpsimd.dma_start(out=gwa, in_=warm_w.bitcast(F32)[0:1, 0:16])
            nc.gpsimd.partition_broadcast(gwb, gwa)
            nc.gpsimd.dma_start(out=gwa, in_=gwb[0:1, :])
            nc.gpsimd.partition_broadcast(gwb, gwa)

            # ---- projections ----
            with tc.tile_pool(name="psproj", bufs=8, space="PSUM") as psproj:
                wv_sb = []
                for k in range(NK):
                    t_ = wpool.tile([128, MODEL_DIM], BF16, tag="w", name=f"wv_{k}")
                    nc.scalar.dma_start(out=t_, in_=wvT[k * 128:(k + 1) * 128, :])
                    wv_sb.append(t_)

                # Q and K projections: psq[p] = w[:, 128p:+128].T @ x.T
                # psq[p][64*half+d, b*128+r]: feature j=128p+64half+d of
                # x-row (block b, r); head h = b; jc = 2p+half.
                # p-outer: each psq[p] finishes its k-accumulation and drains
                # while the PE moves on, so Q/K/V matmul streams stay dense
                for w_sb, bias_sb, is_q in ((wq_sb, bq_sb, True),
                                            (wk_sb, bk_sb, False)):
                    psq = {}
                    for p in range(8):
                        psq[p] = psproj.tile([128, RPC], F32, tag="proj",
                                             name=f"psq_{int(is_q)}_{p}")
                        for k in range(NK):
                            nc.tensor.matmul(
                                psq[p], w_sb[k][:, p * 128:(p + 1) * 128], xt[k],
                                start=(k == 0), stop=(k == NK - 1),
                            )
                        for half in range(2):
                            jc = 2 * p + half
                            for ph in range(2):
                                # src: psq[p][64half+d, (2m+ph)*128+r], m in {0,1}
                                src3 = psq[p][64 * half:64 * half + 64, ph * 128:] \
                                    .rearrange("p (b r) -> p b r", r=128)[:, 0:3:2, :]
                                if is_q:
                                    base = (2 * (jc // 4) + ph) * 512 + (jc % 4) * 128
                                    dst3 = qT2pad[64 * ph:64 * ph + 64, :].rearrange(
                                        "p (m c) -> p m c", m=2)[:, :, base:base + 128]
                                else:
                                    base = jc * 128
                                    dst3 = kT2a[64 * ph:64 * ph + 64, :].rearrange(
                                        "p (m c) -> p m c", m=2)[:, :, base:base + 128]
                                nc.vector.tensor_scalar_add(
                                    dst3, src3,
                                    bias_sb[64 * half:64 * half + 64, p:p + 1],
                                )

                # V projection, bl-outer so v_aug[bl] drains progressively:
                # psv[2bl+jw][r, jj] = feature jw*512+jj of x-row (bl, r)
                for bl in range(4):
                    psv = [psproj.tile([128, RPC], F32, tag="proj",
                                       name=f"psv_{bl}_{i}") for i in range(2)]
                    for k in range(NK):
                        for jw in range(2):
                            nc.tensor.matmul(
                                psv[jw], xt[k][:, bl * 128:(bl + 1) * 128],
                                wv_sb[k][:, jw * 512:(jw + 1) * 512],
                                start=(k == 0), stop=(k == NK - 1),
                            )
                    for jw in range(2):
                        nc.vector.tensor_tensor(
                            v_aug[bl][:, 8 * jw:8 * jw + 8, 0:64],
                            psv[jw][:, :].rearrange(
                                "p (cc d) -> p cc d", d=64),
                            bv_bc[:, jw * 512:(jw + 1) * 512].rearrange(
                                "p (cc d) -> p cc d", d=64),
                            mybir.AluOpType.add,
                        )

            # ---- attention ----
            with (
                tc.tile_pool(name="psS", bufs=2, space="PSUM") as psS_pool,
                tc.tile_pool(name="psO", bufs=1, space="PSUM") as psO_pool,
                tc.tile_pool(name="psF", bufs=1, space="PSUM") as psF_pool,
            ):
                state = {}

                def emit_scores(i, wq, m, cc):
                    """scores MM (N=1024, both halves) + exp -> eS tile."""
                    psS = psS_pool.tile([128, 1024], F32, tag="s", bufs=2,
                                        name=f"psS_{i}")
                    lhs = kT2a[:, m * T + cc * 128: m * T + cc * 128 + 128]
                    base = (m * 4 + wq) * 1024
                    # one PSUM bank per MM: N<=512 fp32 out
                    nc.tensor.matmul(psS[:, 0:512], lhs,
                                     qT2pad[:, base:base + 512],
                                     start=True, stop=True)
                    nc.tensor.matmul(psS[:, 512:1024], lhs,
                                     qT2pad[:, base + 512:base + 1024],
                                     start=True, stop=True)
                    eS = espool.tile([128, 1024], I16, tag="es", bufs=4,
                                     name=f"eS_{i}")
                    if DVE_EXP and (i % 2) == 1:
                        nc.vector.tensor_scalar(
                            eS, psS, float(SC_A), float(SC_B),
                            mybir.AluOpType.mult, mybir.AluOpType.add)
                    else:
                        nc.scalar.activation(
                            eS.bitcast(BF16), psS,
                            mybir.ActivationFunctionType.Exp, scale=0.125)
                    return eS

                def normalize_pair(m, wq, psO_A, psO_B):
                    """psO_{A,B} [65,512] -> o2T[m][:, wq*512:+512].
                    Batched so the gpsimd dma->broadcast library switch
                    happens once, and the copies free the psO banks early."""
                    o_sbs, rcps, bcs = [], [], []
                    for ph, psO in ((0, psO_A), (1, psO_B)):
                        o_sb = opool.tile([65, 512], F32, tag=f"osb{ph}",
                                          bufs=2, name=f"osb{ph}_{m}_{wq}")
                        nc.vector.tensor_copy(o_sb, psO)
                        o_sbs.append(o_sb)
                    for ph in range(2):
                        den_t = rcpool.tile([128, 4], F32, tag=f"dent{ph}",
                                            bufs=2)
                        nc.gpsimd.dma_start(
                            out=den_t,
                            in_=o_sbs[ph][64:65, :].rearrange(
                                "a (p i) -> a p i", p=128))
                        rcp_t = rcpool.tile([128, 4], F32, tag=f"rcpt{ph}",
                                            bufs=2)
                        nc.vector.reciprocal(rcp_t, den_t)
                        rcps.append(rcp_t)
                    for ph in range(2):
                        rcp_flat = rcbig.tile([1, 512], F32, tag=f"rcpf{ph}",
                                              bufs=2)
                        nc.gpsimd.dma_start(
                            out=rcp_flat[0:1, :].rearrange(
                                "a (p i) -> a p i", p=128),
                            in_=rcps[ph])
                        bcs.append(rcp_flat)
                    for ph in range(2):
                        rcp_bc = rcbig.tile([64, 512], F32, tag=f"rcpb{ph}",
                                            bufs=2)
                        nc.gpsimd.partition_broadcast(rcp_bc, bcs[ph])
                        nc.vector.tensor_tensor(
                            o2T[m][64 * ph:64 * ph + 64,
                                   wq * 512:(wq + 1) * 512],
                            o_sbs[ph][0:64, :], rcp_bc, mybir.AluOpType.mult)

                def emit_pv(i, wq, m, cc, eS):
                    if cc == 0:
                        state[(wq, m)] = (
                            psO_pool.tile([65, 512], F32, tag="oA",
                                          name=f"psO_A_{wq}_{m}"),
                            psO_pool.tile([65, 512], F32, tag="oB",
                                          name=f"psO_B_{wq}_{m}"),
                        )
                    psO_A, psO_B = state[(wq, m)]
                    eSb = eS.bitcast(BF16)
                    nc.tensor.matmul(psO_A, v_aug[2 * m][:, cc, :],
                                     eSb[:, 0:512],
                                     start=(cc == 0), stop=(cc == 15))
                    nc.tensor.matmul(psO_B, v_aug[2 * m + 1][:, cc, :],
                                     eSb[:, 512:1024],
                                     start=(cc == 0), stop=(cc == 15))
                    if cc == 15:
                        normalize_pair(m, wq, psO_A, psO_B)

                def emit_final(tt, eng):
                    """outp rows [tt*128, +128): both pairs accumulated.
                    Two 1-bank psF halves; the two PSUM->SBUF copies run on
                    ACT and DVE in parallel and pipeline against the MMs."""
                    psh = [psF_pool.tile([128, 512], F32, tag=f"f{j}",
                                         name=f"psF_{tt}_{j}") for j in range(2)]
                    for jw in range(2):
                        for m2 in range(2):
                            nc.tensor.matmul(
                                psh[jw],
                                o2T[m2][:, tt * 128:(tt + 1) * 128],
                                wo_sb[m2][:, jw * 512:(jw + 1) * 512],
                                start=(m2 == 0), stop=(m2 == 1),
                            )
                    out_sb = outpool.tile([128, MODEL_DIM], F32, tag="out",
                                          bufs=4)
                    nc.vector.tensor_copy(out_sb[:, 0:512], psh[0])
                    nc.vector.tensor_copy(out_sb[:, 512:1024], psh[1])
                    nc.sync.dma_start(
                        out=outp[tt * 128:(tt + 1) * 128, :], in_=out_sb)

                iters = [(wq, m, cc) for wq in range(4) for m in range(2)
                         for cc in range(16)]
                # software pipeline: PV lags scores by 2 so the exp result is
                # ready before the PE reaches the PV matmuls
                pend = []
                nfin = 0
                for i, (wq, m, cc) in enumerate(iters):
                    eS = emit_scores(i, wq, m, cc)
                    pend.append((i, wq, m, cc, eS))
                    if len(pend) > 2:
                        emit_pv(*pend.pop(0))
                    if pend and pend[0][3] == 15:
                        # flush the group-closing PV one iteration early so
                        # its normalize chain starts sooner
                        emit_pv(*pend.pop(0))
                    # finals for wq-1 spread with slack for the normalize chain
                    if wq >= 1 and ((m == 0 and cc in (6, 10, 14)) or
                                    (m == 1 and cc == 2)):
                        emit_final(4 * (wq - 1) + (nfin % 4), nfin % 2)
                        nfin += 1
                for p_ in pend:
                    emit_pv(*p_)
                # dummy matmuls keep the PE array active (HAM warm) while the
                # last group's normalize chain runs, so the trailing finals
                # execute at the warm clock
                psd = psF_pool.tile([128, 512], F32, tag="f0", name="psd")
                for _ in range(14):
                    nc.tensor.matmul(psd, warm_w[:, 0:128], warm_w,
                                     start=True, stop=True)
                for j in range(4):
                    emit_final(12 + j, j % 2)

    nc.compile()
    return nc


_NC_CACHE = None


def _get_program():
    global _NC_CACHE
    if _NC_CACHE is None:
        _NC_CACHE = _build_program()
    return _NC_CACHE


def _bf16(a: np.ndarray) -> np.ndarray:
    return np.asarray(a, np.float32).astype(ml_dtypes.bfloat16)


def _host_prep(inputs):
    x = np.asarray(inputs["x"], np.float32)
    wq = np.asarray(inputs["wq"], np.float32)
    wk = np.asarray(inputs["wk"], np.float32)
    wv = np.asarray(inputs["wv"], np.float32)
    wo = np.asarray(inputs["wo"], np.float32)
    bq = np.asarray(inputs["bq"], np.float32)
    bk = np.asarray(inputs["bk"], np.float32)
    bv = np.asarray(inputs["bv"], np.float32)
    rot_cos = np.asarray(inputs["rot_cos"], np.float32)
    rot_sin = np.asarray(inputs["rot_sin"], np.float32)

    cos = rot_cos[SEQ_POS]
    sin = rot_sin[SEQ_POS]

    def rope_fold_w(w):
        wv_ = w.reshape(16, 32, 2, MODEL_DIM)
        ev = wv_[:, :, 0] * cos[None, :, None] - wv_[:, :, 1] * sin[None, :, None]
        od = wv_[:, :, 0] * sin[None, :, None] + wv_[:, :, 1] * cos[None, :, None]
        return np.stack([ev, od], axis=2).reshape(MODEL_DIM, MODEL_DIM)

    def rope_fold_b(b_):
        bv_ = b_.reshape(16, 32, 2)
        ev = bv_[:, :, 0] * cos[None, :] - bv_[:, :, 1] * sin[None, :]
        od = bv_[:, :, 0] * sin[None, :] + bv_[:, :, 1] * cos[None, :]
        return np.stack([ev, od], axis=2).reshape(MODEL_DIM)

    wq_r = rope_fold_w(wq)
    wk_r = rope_fold_w(wk)
    bq_r = rope_fold_b(bq)
    bk_r = rope_fold_b(bk)

    wqT = _bf16(np.ascontiguousarray(wq_r.T))
    wkT = _bf16(np.ascontiguousarray(wk_r.T))
    wvT = _bf16(np.ascontiguousarray(wv.T))
    bq_sb = np.ascontiguousarray(bq_r.reshape(8, 128).T)
    bk_sb = np.ascontiguousarray(bk_r.reshape(8, 128).T)

    in_maps = []
    for cid in range(N_CORES):
        bi, g = cid // 4, cid % 4
        xTc = _bf16(np.ascontiguousarray(x[bi, 512 * g:512 * (g + 1), :].T))
        woTc = np.stack(
            [np.ascontiguousarray(
                wo[:, (4 * g + 2 * m) * 64:(4 * g + 2 * m + 2) * 64].T)
             for m in range(2)])
        in_maps.append({
            "xT": xTc,
            "wqT": wqT, "wkT": wkT, "wvT": wvT,
            "woT": _bf16(woTc),
            "bq": bq_sb, "bk": bk_sb, "bv": bv,
        })
    return in_maps, np.asarray(inputs["bo"], np.float32)


def _gather(results, bo):
    out = np.empty((B, T, MODEL_DIM), np.float32)
    for bi in range(B):
        acc = results[4 * bi]["outp"].astype(np.float32)
        for g in range(1, 4):
            acc = acc + results[4 * bi + g]["outp"]
        # t'' = jc*128 + r  ->  t = r*16 + jc
        acc = acc.reshape(16, 128, MODEL_DIM).transpose(1, 0, 2).reshape(
            T, MODEL_DIM)
        out[bi] = acc + bo[None, :]
    return out


def _run(inputs, trace=False, **kw):
    nc = _get_program()
    in_maps, bo = _host_prep(inputs)
    res = run_bass_kernel_spmd(nc, in_maps, list(range(N_CORES)), trace=trace,
                               **kw)
    return _gather(res.results, bo), res


def kernel(**inputs) -> np.ndarray:
    out, _ = _run(inputs)
    return out
